# revision 1
# baseline (speedup 1.0000x reference)
"""Contrastive-loss Trainium2 kernel: 8-way data-parallel over similarity rows.

Strategy (per sharding hint): each of the 8 NeuronCores computes a
[1024, 8192] block of the similarity matrix sim = e @ e.T / T against the
full embedding matrix, reduces per-row numerator / denominator / validity
on-device, and returns per-partition partial (loss_sum, valid_count); the
host sums the 8x[128,2] partials.

Key layout trick: rows are sorted by label on the host and each core's
input is rolled so its 1024 rows sit at a fixed offset (PAD). Same-label
columns of any 128-row tile then live in a fixed 640-wide window
[t*128, t*128+640), so the label-mask / positive-gate / numerator work
touches 640 instead of 8192 columns per row. The denominator row-sum comes
free from the Exp activation's accum_out. Matmuls run in bf16 (fp32 PSUM
accumulate); everything downstream of exp is fp32.
"""

import contextlib
import ctypes
import os
import sys
import types

import ml_dtypes
import numpy as np

import concourse.bass as bass
import concourse.mybir as mybir
import concourse.tile as tile
from concourse.bass_utils import run_bass_kernel_spmd

# problem constants (hardcoded per task contract)
N, D, NCLS = 8192, 512, 512
TEMP = 0.07
EPS = 1e-8
M = 8            # cores
R = N // M       # 1024 rows per core
NT = R // 128    # 8 row-tiles per core
PAD = 256        # roll margin; must exceed max class size
WIN = 128 + 2 * PAD   # 640 col window containing all same-label cols of a tile
CH = 512         # matmul moving-dim chunk (one PSUM bank)
GRP = 2048       # columns per psum group / exp call (4 banks)
NG = N // GRP    # 4 groups
KT = D // 128    # 4 contraction tiles

_AXON_SO = "/opt/axon/libaxon_pjrt.so"

LAST_RESULTS = None   # BassKernelResults of the most recent run (for test.py)


def _install_axon_trace_hook():
    """Provide antenv.axon_hooks (NTFF profiling) if the image lacks it."""
    try:
        from antenv.axon_hooks import get_axon_ntff_profile_hook  # noqa: F401
        return
    except ImportError:
        pass
    if not os.path.exists(_AXON_SO):
        return
    try:
        lib = ctypes.CDLL(_AXON_SO)
    except OSError:
        return
    if not hasattr(lib, "axon_start_nrt_profile"):
        return
    lib.axon_start_nrt_profile.argtypes = [ctypes.POINTER(ctypes.c_int64), ctypes.c_size_t]
    lib.axon_start_nrt_profile.restype = ctypes.c_int64
    lib.axon_stop_nrt_profile.argtypes = [ctypes.c_char_p]
    lib.axon_stop_nrt_profile.restype = ctypes.c_int64

    @contextlib.contextmanager
    def _hook(output_dir, device_ids):
        import jax
        jax.devices()
        if device_ids:
            ids = (ctypes.c_int64 * len(device_ids))(*device_ids)
            rc = lib.axon_start_nrt_profile(ids, len(device_ids))
        else:
            rc = lib.axon_start_nrt_profile(None, 0)
        if rc != 0:
            raise RuntimeError(f"axon_start_nrt_profile rc={rc}")
        try:
            yield
        finally:
            n = lib.axon_stop_nrt_profile(str(output_dir).encode())
            if n < 0:
                raise RuntimeError(f"axon_stop_nrt_profile rc={n}")

    _the_hook = [_hook]
    mod = types.ModuleType("antenv.axon_hooks")
    mod.set_axon_ntff_profile_hook = lambda h: _the_hook.__setitem__(0, h)
    mod.get_axon_ntff_profile_hook = lambda: _the_hook[0]
    sys.modules["antenv.axon_hooks"] = mod
    import antenv
    antenv.axon_hooks = mod


def _split_excess_waits(nc, max_waits=1):
    """This walrus build allows one sync-wait per instruction; move extras
    onto same-engine NoOps inserted just before (execution order preserved)."""
    for f in nc.m.functions:
        for b in f.blocks:
            insts = b.instructions
            new = []
            changed = False
            for inst in insts:
                si = inst.sync_info
                ow = list(si.on_wait) if (si and si.on_wait) else []
                if len(ow) > max_waits:
                    extra, keep = ow[:-max_waits], ow[-max_waits:]
                    for k, w in enumerate(extra):
                        nop = mybir.InstNoOp(name=f"{inst.name}-w{k}", ins=[], outs=[])
                        nop.engine = inst.engine
                        nop.sync_info = mybir.SyncInfo(on_wait=[w], on_update=[])
                        new.append(nop)
                    inst.sync_info = mybir.SyncInfo(
                        on_wait=keep,
                        on_update=list(si.on_update) if si.on_update else [])
                    changed = True
                new.append(inst)
            if changed:
                b.instructions = new


def _build_nc():
    f32 = mybir.dt.float32
    bf16 = mybir.dt.bfloat16
    Alu = mybir.AluOpType
    Act = mybir.ActivationFunctionType

    nc = bass.Bass(trn_type="TRN2", target_bir_lowering=False, debug=False)
    qT = nc.dram_tensor("qT", [128, (N // CH) * KT * CH], bf16, kind="ExternalInput")
    labd = nc.dram_tensor("lab", [N, 1], f32, kind="ExternalInput")
    antid = nc.dram_tensor("anti", [128, 128], f32, kind="ExternalInput")
    identd = nc.dram_tensor("ident", [128, 128], f32, kind="ExternalInput")
    outd = nc.dram_tensor("out", [128, 2], f32, kind="ExternalOutput")

    with tile.TileContext(nc) as tc, contextlib.ExitStack() as ctx:
        qp = ctx.enter_context(tc.tile_pool(name="qp", bufs=1))
        pp = ctx.enter_context(tc.tile_pool(name="pp", bufs=2, space="PSUM"))
        ep = ctx.enter_context(tc.tile_pool(name="ep", bufs=3))
        wp = ctx.enter_context(tc.tile_pool(name="wp", bufs=2))
        sp = ctx.enter_context(tc.tile_pool(name="sp", bufs=1))

        # ---- preload ----
        # qT chunks: [128, KT, CH] bf16, one per 512-col chunk
        qt = []
        for n in range(N // CH):
            q = qp.tile([128, KT, CH], bf16, tag=f"q{n}")
            nc.sync.dma_start(
                out=q, in_=qT[:, n * KT * CH:(n + 1) * KT * CH])
            qt.append(q)
        # row labels per (partition, tile): lab[PAD + t*128 + p]
        lab_rows = sp.tile([128, NT, 1], f32)
        nc.sync.dma_start(
            out=lab_rows,
            in_=labd[PAD:PAD + R, :].rearrange("(t p) o -> p t o", p=128))
        # column labels broadcast to all partitions, cols [0, NT*128+WIN)
        labw_w = (NT - 1) * 128 + WIN        # 1536
        labw = sp.tile([128, labw_w], f32)
        nc.sync.dma_start(
            out=labw,
            in_=bass.AP(tensor=labd, offset=0, ap=[[0, 128], [1, labw_w]]))
        anti = sp.tile([128, 128], f32)
        nc.sync.dma_start(out=anti, in_=antid.ap())
        ident = sp.tile([128, 128], f32)
        nc.sync.dma_start(out=ident, in_=identd.ap())
        eps_t = sp.tile([128, 1], f32)
        nc.vector.memset(eps_t, EPS)
        warm = sp.tile([128, 128], bf16)
        nc.vector.memset(warm, 0.0)
        warm_ps = pp.tile([128, GRP], f32, tag="ps")
        for w in range(48):
            nc.tensor.matmul(warm_ps[:, :128], warm, warm, start=True, stop=True)

        # ---- accumulators ----
        dacc = sp.tile([128, NT * NG], f32)   # exp row-sums per (t, g)
        nacc = sp.tile([128, NT], f32)        # numerator per t
        edacc = sp.tile([128, NT], f32)       # diagonal exp per t

        # ---- main loop ----
        for t in range(NT):
            a = (PAD + t * 128) // CH        # lhsT chunk index
            off = (PAD + t * 128) % CH       # lhsT offset within chunk
            for g in range(NG):
                ps = pp.tile([128, GRP], f32, tag="ps")
                for sub in range(GRP // CH):
                    n = g * (GRP // CH) + sub
                    for k in range(KT):
                        nc.tensor.matmul(
                            ps[:, sub * CH:(sub + 1) * CH],
                            qt[a][:, k, off:off + 128],
                            qt[n][:, k, :],
                            start=(k == 0), stop=(k == KT - 1))
                e = ep.tile([128, GRP], f32, tag="e")
                nc.scalar.activation(
                    out=e, in_=ps[:], func=Act.Exp, scale=float(1.0 / TEMP),
                    accum_out=dacc[:, t * NG + g:t * NG + g + 1])
                if g == 0:
                    # window = cols [t*128, t*128+WIN) -- inside group 0
                    w0 = t * 128
                    u = wp.tile([128, WIN], f32, tag="u")
                    # u = (lab_col == lab_row) * exp(sim)
                    nc.vector.scalar_tensor_tensor(
                        out=u, in0=labw[:, w0:w0 + WIN],
                        scalar=lab_rows[:, t, :], in1=e[:, w0:w0 + WIN],
                        op0=Alu.is_equal, op1=Alu.mult)
                    # diagonal sits at window cols [PAD, PAD+128)
                    scr = wp.tile([128, 128], f32, tag="scr")
                    nc.vector.scalar_tensor_tensor(
                        out=scr, in0=u[:, PAD:PAD + 128], scalar=1.0,
                        in1=ident, op0=Alu.mult, op1=Alu.mult,
                        accum_out=edacc[:, t:t + 1])
                    nc.vector.tensor_tensor(
                        out=u[:, PAD:PAD + 128], in0=u[:, PAD:PAD + 128],
                        in1=anti, op=Alu.mult)
                    # numerator: sum over u where u > 1  (sim>0 gate)
                    scr2 = wp.tile([128, WIN], f32, tag="scr2")
                    nc.vector.scalar_tensor_tensor(
                        out=scr2, in0=u, scalar=1.0, in1=u,
                        op0=Alu.is_gt, op1=Alu.mult,
                        accum_out=nacc[:, t:t + 1])

        # ---- epilogue (all [128, NT]) ----
        dred = sp.tile([128, NT], f32)
        nc.vector.tensor_reduce(
            out=dred, in_=dacc.rearrange("p (t g) -> p t g", g=NG),
            axis=mybir.AxisListType.X, op=Alu.add)
        den = sp.tile([128, NT], f32)
        nc.vector.tensor_tensor(out=den, in0=dred, in1=edacc, op=Alu.subtract)
        v1 = sp.tile([128, NT], f32)
        nc.vector.tensor_scalar(out=v1, in0=nacc, scalar1=0.0, scalar2=None,
                                op0=Alu.is_gt)
        v2 = sp.tile([128, NT], f32)
        nc.vector.tensor_scalar(out=v2, in0=den, scalar1=0.0, scalar2=None,
                                op0=Alu.is_gt)
        v = sp.tile([128, NT], f32)
        nc.vector.tensor_tensor(out=v, in0=v1, in1=v2, op=Alu.mult)
        inv = sp.tile([128, NT], f32)
        nc.vector.tensor_scalar(out=inv, in0=v, scalar1=0.0, scalar2=None,
                                op0=Alu.is_equal)
        nsafe = sp.tile([128, NT], f32)
        nc.vector.tensor_tensor(out=nsafe, in0=nacc, in1=v, op=Alu.mult)
        nc.vector.tensor_tensor(out=nsafe, in0=nsafe, in1=inv, op=Alu.add)
        dsafe = sp.tile([128, NT], f32)
        nc.vector.tensor_tensor(out=dsafe, in0=den, in1=v, op=Alu.mult)
        nc.vector.tensor_tensor(out=dsafe, in0=dsafe, in1=inv, op=Alu.add)
        lgd = sp.tile([128, NT], f32)
        nc.scalar.activation(out=lgd, in_=dsafe, func=Act.Ln, bias=eps_t[:], scale=1.0)
        lgn = sp.tile([128, NT], f32)
        nc.scalar.activation(out=lgn, in_=nsafe, func=Act.Ln, scale=1.0)
        li = sp.tile([128, NT], f32)
        nc.vector.tensor_tensor(out=li, in0=lgd, in1=lgn, op=Alu.subtract)
        nc.vector.tensor_tensor(out=li, in0=li, in1=v, op=Alu.mult)
        o = sp.tile([128, 2], f32)
        nc.vector.tensor_reduce(out=o[:, 0:1], in_=li, axis=mybir.AxisListType.X,
                                op=Alu.add)
        nc.vector.tensor_reduce(out=o[:, 1:2], in_=v, axis=mybir.AxisListType.X,
                                op=Alu.add)
        nc.sync.dma_start(out=outd.ap(), in_=o)

    _split_excess_waits(nc)
    return nc


_NC = None


def _get_nc():
    global _NC
    if _NC is None:
        _NC = _build_nc()
    return _NC


def _host_reference(emb, lab):
    """Numpy fallback (only for pathological label distributions where a
    class exceeds the PAD margin; never triggers for the target regime)."""
    e = emb / np.linalg.norm(emb, axis=1, keepdims=True).astype(np.float32)
    sim = (e @ e.T).astype(np.float32) / np.float32(TEMP)
    E = np.exp(sim, dtype=np.float32)
    pos = (lab[:, None] == lab[None, :]) & ~np.eye(len(lab), dtype=bool)
    valid = pos & (sim > 0)
    num = np.where(valid, E, 0).sum(1, dtype=np.float32)
    den = E.sum(1, dtype=np.float32) - np.diagonal(E)
    rv = valid.any(1) & (den > 0)
    ns = np.where(rv, num, np.float32(1.0))
    ds = np.where(rv, den, np.float32(1.0))
    li = np.log(ds + np.float32(EPS)) - np.log(ns)
    nv = int(rv.sum())
    if nv == 0:
        return np.float32(0.0)
    return np.float32(abs(float(np.where(rv, li, 0).sum(dtype=np.float64)) / nv))


def kernel(**inputs):
    global LAST_RESULTS
    emb = np.ascontiguousarray(np.asarray(inputs["embeddings"], dtype=np.float32))
    lab = np.asarray(inputs["labels"]).astype(np.int64).ravel()
    assert emb.shape == (N, D) and lab.shape == (N,)

    if np.bincount(lab, minlength=1).max() > PAD:
        return _host_reference(emb, lab)

    _install_axon_trace_hook()

    # host prep: normalize, sort by label, per-core roll + transpose
    e = emb / np.linalg.norm(emb, axis=1, keepdims=True).astype(np.float32)
    order = np.argsort(lab, kind="stable")
    es = np.ascontiguousarray(e[order])
    ls = lab[order].astype(np.float32)

    anti = (1.0 - np.eye(128, dtype=np.float32)).astype(np.float32)
    ident = np.eye(128, dtype=np.float32)

    in_maps = []
    for c in range(M):
        shift = c * R - PAD
        rolled = np.roll(es, -shift, axis=0)         # [N, D] f32
        labr = np.roll(ls, -shift).reshape(N, 1)     # [N, 1] f32
        # [D, N] -> [128, NCH, KT, CH]: partition p, chunk n holds
        # qT[k*128+p, n*CH:(n+1)*CH] contiguckus per (k)
        qTc = (rolled.T.reshape(KT, 128, N // CH, CH)
               .transpose(1, 2, 0, 3)
               .reshape(128, (N // CH) * KT * CH)
               .astype(ml_dtypes.bfloat16))
        qTc = np.ascontiguousarray(qTc)
        in_maps.append({
            "qT": qTc,
            "lab": np.ascontiguousarray(labr),
            "anti": anti,
            "ident": ident,
        })

    nc = _get_nc()
    res = run_bass_kernel_spmd(nc, in_maps, core_ids=list(range(M)))
    LAST_RESULTS = res

    loss_sum = 0.0
    cnt = 0.0
    for c in range(M):
        o = res.results[c]["out"]
        loss_sum += float(o[:, 0].sum(dtype=np.float64))
        cnt += float(o[:, 1].sum(dtype=np.float64))
    if cnt <= 0:
        return np.float32(0.0)
    return np.float32(abs(loss_sum / cnt))



# revision 6
# speedup vs baseline: 1.4603x; 1.4603x over previous
"""Contrastive-loss Trainium2 kernel: 8-way data-parallel over similarity rows.

Strategy (per sharding hint): each of the 8 NeuronCores computes a
[1024, 8192] block of the similarity matrix sim = e @ e.T / T against the
full embedding matrix, reduces per-row numerator / denominator / validity
on-device, and returns per-partition partial (loss_sum, valid_count); the
host sums the 8x[128,2] partials.

Key layout trick: rows are sorted by label on the host and each core's
input is rolled so its 1024 rows sit at a fixed offset (PAD). Same-label
columns of any 128-row tile then live in a fixed 640-wide window
[t*128, t*128+640), so the label-mask / positive-gate / numerator work
touches 640 instead of 8192 columns per row. The denominator row-sum comes
free from the Exp activation's accum_out. Matmuls run in bf16 (fp32 PSUM
accumulate); everything downstream of exp is fp32.
"""

import contextlib
import ctypes
import os
import sys
import types

import ml_dtypes
import numpy as np

import concourse.bass as bass
import concourse.mybir as mybir
import concourse.tile as tile
from concourse.bass_utils import run_bass_kernel_spmd

# problem constants (hardcoded per task contract)
N, D, NCLS = 8192, 512, 512
TEMP = 0.07
EPS = 1e-8
M = 8            # cores
R = N // M       # 1024 rows per core
NT = R // 128    # 8 row-tiles per core
PAD = 256        # roll margin; must exceed max class size
WIN = 128 + 2 * PAD   # 640 col window containing all same-label cols of a tile
CH = 512         # matmul moving-dim chunk (one PSUM bank)
GRP = 2048       # columns per psum group / exp call (4 banks)
NG = N // GRP    # 4 groups
KT = D // 128    # 4 contraction tiles
SCL = 8.0        # fp8 pre-quantization scale (power of 2; avoids subnormals)
ESCALE = 1.0 / (SCL * SCL * TEMP)   # exp() input scale un-doing SCL^2

_AXON_SO = "/opt/axon/libaxon_pjrt.so"

LAST_RESULTS = None   # BassKernelResults of the most recent run (for test.py)


def _install_axon_trace_hook():
    """Provide antenv.axon_hooks (NTFF profiling) if the image lacks it."""
    try:
        from antenv.axon_hooks import get_axon_ntff_profile_hook  # noqa: F401
        return
    except ImportError:
        pass
    if not os.path.exists(_AXON_SO):
        return
    try:
        lib = ctypes.CDLL(_AXON_SO)
    except OSError:
        return
    if not hasattr(lib, "axon_start_nrt_profile"):
        return
    lib.axon_start_nrt_profile.argtypes = [ctypes.POINTER(ctypes.c_int64), ctypes.c_size_t]
    lib.axon_start_nrt_profile.restype = ctypes.c_int64
    lib.axon_stop_nrt_profile.argtypes = [ctypes.c_char_p]
    lib.axon_stop_nrt_profile.restype = ctypes.c_int64

    @contextlib.contextmanager
    def _hook(output_dir, device_ids):
        import jax
        jax.devices()
        if device_ids:
            ids = (ctypes.c_int64 * len(device_ids))(*device_ids)
            rc = lib.axon_start_nrt_profile(ids, len(device_ids))
        else:
            rc = lib.axon_start_nrt_profile(None, 0)
        if rc != 0:
            raise RuntimeError(f"axon_start_nrt_profile rc={rc}")
        try:
            yield
        finally:
            n = lib.axon_stop_nrt_profile(str(output_dir).encode())
            if n < 0:
                raise RuntimeError(f"axon_stop_nrt_profile rc={n}")

    _the_hook = [_hook]
    mod = types.ModuleType("antenv.axon_hooks")
    mod.set_axon_ntff_profile_hook = lambda h: _the_hook.__setitem__(0, h)
    mod.get_axon_ntff_profile_hook = lambda: _the_hook[0]
    sys.modules["antenv.axon_hooks"] = mod
    import antenv
    antenv.axon_hooks = mod


def _split_excess_waits(nc, max_waits=1):
    """This walrus build allows one sync-wait per instruction; move extras
    onto same-engine NoOps inserted just before (execution order preserved)."""
    for f in nc.m.functions:
        for b in f.blocks:
            insts = b.instructions
            new = []
            changed = False
            for inst in insts:
                si = inst.sync_info
                ow = list(si.on_wait) if (si and si.on_wait) else []
                if len(ow) > max_waits:
                    extra, keep = ow[:-max_waits], ow[-max_waits:]
                    for k, w in enumerate(extra):
                        nop = mybir.InstNoOp(name=f"{inst.name}-w{k}", ins=[], outs=[])
                        nop.engine = inst.engine
                        nop.sync_info = mybir.SyncInfo(on_wait=[w], on_update=[])
                        new.append(nop)
                    inst.sync_info = mybir.SyncInfo(
                        on_wait=keep,
                        on_update=list(si.on_update) if si.on_update else [])
                    changed = True
                new.append(inst)
            if changed:
                b.instructions = new


def _build_nc():
    f32 = mybir.dt.float32
    bf16 = mybir.dt.bfloat16
    f8 = mybir.dt.float8e4
    Alu = mybir.AluOpType
    Act = mybir.ActivationFunctionType
    DR = mybir.MatmulPerfMode.DoubleRow

    nc = bass.Bass(trn_type="TRN2", target_bir_lowering=False, debug=False)
    qT = nc.dram_tensor("qT", [128, KT * N], f8, kind="ExternalInput")
    labd = nc.dram_tensor("lab", [N, 1], f32, kind="ExternalInput")
    antid = nc.dram_tensor("anti", [128, 128], f32, kind="ExternalInput")
    identd = nc.dram_tensor("ident", [128, 128], f32, kind="ExternalInput")
    outd = nc.dram_tensor("out", [128, 2], f32, kind="ExternalOutput")

    with tile.TileContext(nc) as tc, contextlib.ExitStack() as ctx:
        qp = ctx.enter_context(tc.tile_pool(name="qp", bufs=1))
        pp = ctx.enter_context(tc.tile_pool(name="pp", bufs=2, space="PSUM"))
        ep = ctx.enter_context(tc.tile_pool(name="ep", bufs=3))
        wp = ctx.enter_context(tc.tile_pool(name="wp", bufs=2))
        sp = ctx.enter_context(tc.tile_pool(name="sp", bufs=1))

        # ---- preload ----
        # qT: [128, KT, N] fp8 (k-subtile major, full col range contiguous)
        qt = qp.tile([128, KT, N], f8, tag="qt")
        nc.sync.dma_start(out=qt, in_=qT[:, :])
        # row labels per (partition, tile): lab[PAD + t*128 + p]
        lab_rows = sp.tile([128, NT, 1], f32)
        nc.sync.dma_start(
            out=lab_rows,
            in_=labd[PAD:PAD + R, :].rearrange("(t p) o -> p t o", p=128))
        # column labels broadcast to all partitions, cols [0, NT*128+WIN)
        labw_w = (NT - 1) * 128 + WIN        # 1536
        labw = sp.tile([128, labw_w], f32)
        nc.sync.dma_start(
            out=labw,
            in_=bass.AP(tensor=labd, offset=0, ap=[[0, 128], [1, labw_w]]))
        anti = sp.tile([128, 128], f32)
        nc.sync.dma_start(out=anti, in_=antid.ap())
        ident = sp.tile([128, 128], f32)
        nc.sync.dma_start(out=ident, in_=identd.ap())
        eps_t = sp.tile([128, 1], f32)
        nc.vector.memset(eps_t, EPS)
        warm = sp.tile([128, 128], bf16)
        nc.vector.memset(warm, 0.0)
        warm_ps = pp.tile([128, GRP], f32, tag="ps")
        for w in range(48):
            nc.tensor.matmul(warm_ps[:, :128], warm, warm, start=True, stop=True)

        # ---- accumulators ----
        dacc = sp.tile([128, NT * NG], f32)   # exp row-sums per (t, g)
        nacc = sp.tile([128, NT], f32)        # numerator per t
        edacc = sp.tile([128, NT], f32)       # diagonal exp per t

        # ---- main loop ----
        for t in range(NT):
            off = PAD + t * 128              # lhsT col offset in qt
            for g in range(NG):
                ps = pp.tile([128, GRP], f32, tag="ps")
                for sub in range(GRP // CH):
                    c0 = g * GRP + sub * CH
                    for k in range(0, KT, 2):
                        nc.tensor.matmul(
                            ps[:, sub * CH:(sub + 1) * CH],
                            qt[:, k:k + 2, off:off + 128],
                            qt[:, k:k + 2, c0:c0 + CH],
                            start=(k == 0), stop=(k == KT - 2),
                            perf_mode=DR)
                e = ep.tile([128, GRP], f32, tag="e")
                nc.scalar.activation(
                    out=e, in_=ps[:], func=Act.Exp, scale=float(ESCALE),
                    accum_out=dacc[:, t * NG + g:t * NG + g + 1])
                if g == 0:
                    # window = cols [t*128, t*128+WIN) -- inside group 0
                    w0 = t * 128
                    u = wp.tile([128, WIN], f32, tag="u")
                    # u = (lab_col == lab_row) * exp(sim)
                    nc.vector.scalar_tensor_tensor(
                        out=u, in0=labw[:, w0:w0 + WIN],
                        scalar=lab_rows[:, t, :], in1=e[:, w0:w0 + WIN],
                        op0=Alu.is_equal, op1=Alu.mult)
                    # diagonal sits at window cols [PAD, PAD+128)
                    scr = wp.tile([128, 128], f32, tag="scr")
                    nc.vector.scalar_tensor_tensor(
                        out=scr, in0=u[:, PAD:PAD + 128], scalar=1.0,
                        in1=ident, op0=Alu.mult, op1=Alu.mult,
                        accum_out=edacc[:, t:t + 1])
                    nc.vector.tensor_tensor(
                        out=u[:, PAD:PAD + 128], in0=u[:, PAD:PAD + 128],
                        in1=anti, op=Alu.mult)
                    # numerator: sum over u where u > 1  (sim>0 gate)
                    scr2 = wp.tile([128, WIN], f32, tag="scr2")
                    nc.vector.scalar_tensor_tensor(
                        out=scr2, in0=u, scalar=1.0, in1=u,
                        op0=Alu.is_gt, op1=Alu.mult,
                        accum_out=nacc[:, t:t + 1])

        # ---- epilogue (all [128, NT]) ----
        dred = sp.tile([128, NT], f32)
        nc.vector.tensor_reduce(
            out=dred, in_=dacc.rearrange("p (t g) -> p t g", g=NG),
            axis=mybir.AxisListType.X, op=Alu.add)
        den = sp.tile([128, NT], f32)
        nc.vector.tensor_tensor(out=den, in0=dred, in1=edacc, op=Alu.subtract)
        v1 = sp.tile([128, NT], f32)
        nc.vector.tensor_scalar(out=v1, in0=nacc, scalar1=0.0, scalar2=None,
                                op0=Alu.is_gt)
        v2 = sp.tile([128, NT], f32)
        nc.vector.tensor_scalar(out=v2, in0=den, scalar1=0.0, scalar2=None,
                                op0=Alu.is_gt)
        v = sp.tile([128, NT], f32)
        nc.vector.tensor_tensor(out=v, in0=v1, in1=v2, op=Alu.mult)
        inv = sp.tile([128, NT], f32)
        nc.vector.tensor_scalar(out=inv, in0=v, scalar1=0.0, scalar2=None,
                                op0=Alu.is_equal)
        nsafe = sp.tile([128, NT], f32)
        nc.vector.tensor_tensor(out=nsafe, in0=nacc, in1=v, op=Alu.mult)
        nc.vector.tensor_tensor(out=nsafe, in0=nsafe, in1=inv, op=Alu.add)
        dsafe = sp.tile([128, NT], f32)
        nc.vector.tensor_tensor(out=dsafe, in0=den, in1=v, op=Alu.mult)
        nc.vector.tensor_tensor(out=dsafe, in0=dsafe, in1=inv, op=Alu.add)
        lgd = sp.tile([128, NT], f32)
        nc.scalar.activation(out=lgd, in_=dsafe, func=Act.Ln, bias=eps_t[:], scale=1.0)
        lgn = sp.tile([128, NT], f32)
        nc.scalar.activation(out=lgn, in_=nsafe, func=Act.Ln, scale=1.0)
        li = sp.tile([128, NT], f32)
        nc.vector.tensor_tensor(out=li, in0=lgd, in1=lgn, op=Alu.subtract)
        nc.vector.tensor_tensor(out=li, in0=li, in1=v, op=Alu.mult)
        o = sp.tile([128, 2], f32)
        nc.vector.tensor_reduce(out=o[:, 0:1], in_=li, axis=mybir.AxisListType.X,
                                op=Alu.add)
        nc.vector.tensor_reduce(out=o[:, 1:2], in_=v, axis=mybir.AxisListType.X,
                                op=Alu.add)
        nc.sync.dma_start(out=outd.ap(), in_=o)

    _split_excess_waits(nc)
    return nc


_NC = None


def _get_nc():
    global _NC
    if _NC is None:
        _NC = _build_nc()
    return _NC


def _host_reference(emb, lab):
    """Numpy fallback (only for pathological label distributions where a
    class exceeds the PAD margin; never triggers for the target regime)."""
    e = emb / np.linalg.norm(emb, axis=1, keepdims=True).astype(np.float32)
    sim = (e @ e.T).astype(np.float32) / np.float32(TEMP)
    E = np.exp(sim, dtype=np.float32)
    pos = (lab[:, None] == lab[None, :]) & ~np.eye(len(lab), dtype=bool)
    valid = pos & (sim > 0)
    num = np.where(valid, E, 0).sum(1, dtype=np.float32)
    den = E.sum(1, dtype=np.float32) - np.diagonal(E)
    rv = valid.any(1) & (den > 0)
    ns = np.where(rv, num, np.float32(1.0))
    ds = np.where(rv, den, np.float32(1.0))
    li = np.log(ds + np.float32(EPS)) - np.log(ns)
    nv = int(rv.sum())
    if nv == 0:
        return np.float32(0.0)
    return np.float32(abs(float(np.where(rv, li, 0).sum(dtype=np.float64)) / nv))


def kernel(**inputs):
    global LAST_RESULTS
    emb = np.ascontiguousarray(np.asarray(inputs["embeddings"], dtype=np.float32))
    lab = np.asarray(inputs["labels"]).astype(np.int64).ravel()
    assert emb.shape == (N, D) and lab.shape == (N,)

    if np.bincount(lab, minlength=1).max() > PAD:
        return _host_reference(emb, lab)

    _install_axon_trace_hook()

    # host prep: normalize, sort by label, per-core roll + transpose
    e = emb / np.linalg.norm(emb, axis=1, keepdims=True).astype(np.float32)
    order = np.argsort(lab, kind="stable")
    es = np.ascontiguousarray(e[order])
    ls = lab[order].astype(np.float32)

    anti = (1.0 - np.eye(128, dtype=np.float32)).astype(np.float32)
    ident = np.eye(128, dtype=np.float32)

    in_maps = []
    for c in range(M):
        shift = c * R - PAD
        rolled = np.roll(es, -shift, axis=0)         # [N, D] f32
        labr = np.roll(ls, -shift).reshape(N, 1)     # [N, 1] f32
        # [D, N] -> [128, KT, N]: partition p, k-subtile k holds
        # qT[k*128+p, :] (full col range contiguous per k)
        qTc = ((rolled.T * SCL).reshape(KT, 128, N)
               .transpose(1, 0, 2)
               .reshape(128, KT * N)
               .astype(ml_dtypes.float8_e4m3))
        qTc = np.ascontiguousarray(qTc)
        in_maps.append({
            "qT": qTc,
            "lab": np.ascontiguousarray(labr),
            "anti": anti,
            "ident": ident,
        })

    nc = _get_nc()
    res = run_bass_kernel_spmd(nc, in_maps, core_ids=list(range(M)))
    LAST_RESULTS = res

    loss_sum = 0.0
    cnt = 0.0
    for c in range(M):
        o = res.results[c]["out"]
        loss_sum += float(o[:, 0].sum(dtype=np.float64))
        cnt += float(o[:, 1].sum(dtype=np.float64))
    if cnt <= 0:
        return np.float32(0.0)
    return np.float32(abs(loss_sum / cnt))



# revision 7
# speedup vs baseline: 1.5408x; 1.0551x over previous
"""Contrastive-loss Trainium2 kernel: 8-way data-parallel, symmetric-half sims.

Strategy: rows are label-sorted; each of the 8 cores owns 8 row-tiles of 128
rows (its [1024] rows sit at fixed offset PAD in a per-core rolled layout).
Exploiting sim's symmetry, each row-tile computes only a 33-tile (4224-col)
strip starting at its diagonal tile: unordered pairs at circular tile
distance d are covered once for d<32, and the d=32 tile is computed by both
partners at weight 0.5. Per strip: fp8 DoubleRow matmuls (fp32 PSUM), exp on
ScalarE (row-sums via accum_out feed the denominator), bf16 exp tiles
accumulated into a column-sum buffer on DVE (the transposed halves of both
the denominator and the masked positive numerator). Per-core partials
(row/col sums, diag exp) are combined on the host in fp64.
"""

import contextlib
import ctypes
import os
import sys
import types

import ml_dtypes
import numpy as np

import concourse.bass as bass
import concourse.mybir as mybir
import concourse.tile as tile
from concourse.bass_utils import run_bass_kernel_spmd

# problem constants (hardcoded per task contract)
N, D, NCLS = 8192, 512, 512
TEMP = 0.07
EPS = 1e-8
M = 8            # cores
R = N // M       # 1024 rows per core
NT = R // 128    # 8 row-tiles per core
PAD = 256        # roll margin; must exceed max class size
KT = D // 128    # 4 contraction tiles
GRP = 2048       # columns per psum group / exp call (4 banks)
ST = 33          # strip tiles per row-tile (d = 0..32; d=32 at weight 0.5)
SW = ST * 128    # 4224 strip width
CW = PAD + (NT - 1) * 128 + SW   # 5376 colacc width (max strip end)
NW = PAD + (NT - 1) * 128 + 384  # 1536 numcol/label window width
SCL = 8.0        # fp8 pre-quantization scale (power of 2; avoids subnormals)
ESCALE = 1.0 / (SCL * SCL * TEMP)   # exp() input scale un-doing SCL^2
LOFF = 256.0     # label encoding offset: lab-256 in [-256,255] is bf16-exact

_AXON_SO = "/opt/axon/libaxon_pjrt.so"

LAST_RESULTS = None   # BassKernelResults of the most recent run (for test.py)


def _install_axon_trace_hook():
    """Provide antenv.axon_hooks (NTFF profiling) if the image lacks it."""
    try:
        from antenv.axon_hooks import get_axon_ntff_profile_hook  # noqa: F401
        return
    except ImportError:
        pass
    if not os.path.exists(_AXON_SO):
        return
    try:
        lib = ctypes.CDLL(_AXON_SO)
    except OSError:
        return
    if not hasattr(lib, "axon_start_nrt_profile"):
        return
    lib.axon_start_nrt_profile.argtypes = [ctypes.POINTER(ctypes.c_int64), ctypes.c_size_t]
    lib.axon_start_nrt_profile.restype = ctypes.c_int64
    lib.axon_stop_nrt_profile.argtypes = [ctypes.c_char_p]
    lib.axon_stop_nrt_profile.restype = ctypes.c_int64

    @contextlib.contextmanager
    def _hook(output_dir, device_ids):
        import jax
        jax.devices()
        if device_ids:
            ids = (ctypes.c_int64 * len(device_ids))(*device_ids)
            rc = lib.axon_start_nrt_profile(ids, len(device_ids))
        else:
            rc = lib.axon_start_nrt_profile(None, 0)
        if rc != 0:
            raise RuntimeError(f"axon_start_nrt_profile rc={rc}")
        try:
            yield
        finally:
            n = lib.axon_stop_nrt_profile(str(output_dir).encode())
            if n < 0:
                raise RuntimeError(f"axon_stop_nrt_profile rc={n}")

    _the_hook = [_hook]
    mod = types.ModuleType("antenv.axon_hooks")
    mod.set_axon_ntff_profile_hook = lambda h: _the_hook.__setitem__(0, h)
    mod.get_axon_ntff_profile_hook = lambda: _the_hook[0]
    sys.modules["antenv.axon_hooks"] = mod
    import antenv
    antenv.axon_hooks = mod


def _split_excess_waits(nc, max_waits=1):
    """This walrus build allows one sync-wait per instruction; move extras
    onto same-engine NoOps inserted just before (execution order preserved)."""
    for f in nc.m.functions:
        for b in f.blocks:
            insts = b.instructions
            new = []
            changed = False
            for inst in insts:
                si = inst.sync_info
                ow = list(si.on_wait) if (si and si.on_wait) else []
                if len(ow) > max_waits:
                    extra, keep = ow[:-max_waits], ow[-max_waits:]
                    for k, w in enumerate(extra):
                        nop = mybir.InstNoOp(name=f"{inst.name}-w{k}", ins=[], outs=[])
                        nop.engine = inst.engine
                        nop.sync_info = mybir.SyncInfo(on_wait=[w], on_update=[])
                        new.append(nop)
                    inst.sync_info = mybir.SyncInfo(
                        on_wait=keep,
                        on_update=list(si.on_update) if si.on_update else [])
                    changed = True
                new.append(inst)
            if changed:
                b.instructions = new


def _build_nc():
    f32 = mybir.dt.float32
    bf16 = mybir.dt.bfloat16
    f8 = mybir.dt.float8e4
    Alu = mybir.AluOpType
    Act = mybir.ActivationFunctionType
    DR = mybir.MatmulPerfMode.DoubleRow

    nc = bass.Bass(trn_type="TRN2", target_bir_lowering=False, debug=False)
    # k-subtile pairs split into two tensors so matmuls on pair 0 can start
    # once the first half of the embedding DMA lands
    qT01 = nc.dram_tensor("qT01", [128, 2 * N], f8, kind="ExternalInput")
    qT23 = nc.dram_tensor("qT23", [128, 2 * N], f8, kind="ExternalInput")
    labd = nc.dram_tensor("lab", [N, 1], bf16, kind="ExternalInput")
    antid = nc.dram_tensor("anti", [128, 128], f32, kind="ExternalInput")
    identd = nc.dram_tensor("ident", [128, 128], f32, kind="ExternalInput")
    # row-wise partials: dacc[4*NT] | naccA[NT] | naccB[NT] | edacc[NT]
    outd = nc.dram_tensor("out", [128, 6 * NT + NT], f32, kind="ExternalOutput")
    outcold = nc.dram_tensor("outcol", [128, CW], bf16, kind="ExternalOutput")
    outnumd = nc.dram_tensor("outnum", [128, NW], bf16, kind="ExternalOutput")

    with tile.TileContext(nc) as tc, contextlib.ExitStack() as ctx:
        qp = ctx.enter_context(tc.tile_pool(name="qp", bufs=1))
        pp = ctx.enter_context(tc.tile_pool(name="pp", bufs=2, space="PSUM"))
        ep = ctx.enter_context(tc.tile_pool(name="ep", bufs=2))
        wp = ctx.enter_context(tc.tile_pool(name="wp", bufs=2))
        sp = ctx.enter_context(tc.tile_pool(name="sp", bufs=1))

        # ---- preload ----
        qt01 = qp.tile([128, 2, N], f8, tag="qt01")
        nc.sync.dma_start(out=qt01, in_=qT01[:, :])
        qt23 = qp.tile([128, 2, N], f8, tag="qt23")
        nc.sync.dma_start(out=qt23, in_=qT23[:, :])
        # row labels per (partition, tile): lab[PAD + t*128 + p]
        lab_rows = sp.tile([128, NT, 1], bf16)
        nc.sync.dma_start(
            out=lab_rows,
            in_=labd[PAD:PAD + R, :].rearrange("(t p) o -> p t o", p=128))
        # column labels broadcast to all partitions, cols [0, NW)
        labw = sp.tile([128, NW], bf16)
        nc.sync.dma_start(
            out=labw,
            in_=bass.AP(tensor=labd, offset=0, ap=[[0, 128], [1, NW]]))
        anti = sp.tile([128, 128], f32)
        nc.sync.dma_start(out=anti, in_=antid.ap())
        ident = sp.tile([128, 128], f32)
        nc.sync.dma_start(out=ident, in_=identd.ap())
        warm = sp.tile([128, 128], bf16)
        nc.vector.memset(warm, 0.0)
        warm_ps = pp.tile([128, GRP], f32, tag="ps")
        for w in range(48):
            nc.tensor.matmul(warm_ps[:, :128], warm, warm, start=True, stop=True)

        # ---- accumulators ----
        colacc = sp.tile([128, CW], bf16)    # exp col-sums (transposed den)
        nc.vector.memset(colacc, 0.0)
        numcol = sp.tile([128, NW], bf16)    # masked-pos col-sums (transposed num)
        nc.vector.memset(numcol, 0.0)
        dacc = sp.tile([128, 4 * NT], f32)   # exp row-sums: 4 group slots per t
        naccA = sp.tile([128, NT], f32)      # numerator, diag tile
        naccB = sp.tile([128, NT], f32)      # numerator, off-diag window
        edacc = sp.tile([128, NT], f32)      # diagonal exp per t

        def mm_chunk(ps_out, c0, w):
            """accumulate ps_out[128, w] = q[:, rows]^T @ q[:, c0:c0+w]"""
            nc.tensor.matmul(ps_out, qt01[:, :, s0:s0 + 128],
                             qt01[:, :, c0:c0 + w], start=True, stop=False,
                             perf_mode=DR)
            nc.tensor.matmul(ps_out, qt23[:, :, s0:s0 + 128],
                             qt23[:, :, c0:c0 + w], start=False, stop=True,
                             perf_mode=DR)

        # ---- main loop: per row-tile, strip cols [s0, s0+SW) ----
        for t in range(NT):
            s0 = PAD + 128 * t
            # group 0: cols [s0, s0+2048): diag tile (f32) + off-diag (bf16)
            ps0 = pp.tile([128, GRP], f32, tag="ps")
            for sub in range(4):
                mm_chunk(ps0[:, sub * 512:(sub + 1) * 512], s0 + sub * 512, 512)
            ed = ep.tile([128, 128], f32, tag="ed")
            nc.scalar.activation(
                out=ed, in_=ps0[:, 0:128], func=Act.Exp, scale=float(ESCALE),
                accum_out=dacc[:, 4 * t:4 * t + 1])
            e0 = ep.tile([128, GRP - 128], bf16, tag="e0")
            nc.scalar.activation(
                out=e0, in_=ps0[:, 128:GRP], func=Act.Exp, scale=float(ESCALE),
                accum_out=dacc[:, 4 * t + 1:4 * t + 2])
            # group 1: cols [s0+2048, s0+4096)
            ps1 = pp.tile([128, GRP], f32, tag="ps")
            for sub in range(4):
                mm_chunk(ps1[:, sub * 512:(sub + 1) * 512],
                         s0 + GRP + sub * 512, 512)
            e1 = ep.tile([128, GRP], bf16, tag="e1")
            nc.scalar.activation(
                out=e1, in_=ps1[:], func=Act.Exp, scale=float(ESCALE),
                accum_out=dacc[:, 4 * t + 2:4 * t + 3])
            # group 2: cols [s0+4096, s0+4224), weight 0.5 (d=32 pair dup)
            ps2 = pp.tile([128, GRP], f32, tag="ps")
            mm_chunk(ps2[:, 0:128], s0 + 4096, 128)
            e2 = ep.tile([128, 128], bf16, tag="e2")
            nc.scalar.activation(
                out=e2, in_=ps2[:, 0:128], func=Act.Exp, scale=float(ESCALE),
                accum_out=dacc[:, 4 * t + 3:4 * t + 4])

            # column accumulation (DVE, bf16 2x): den transposed halves
            nc.vector.tensor_tensor(
                out=colacc[:, s0 + 128:s0 + GRP],
                in0=colacc[:, s0 + 128:s0 + GRP], in1=e0, op=Alu.add)
            nc.vector.tensor_tensor(
                out=colacc[:, s0 + GRP:s0 + 2 * GRP],
                in0=colacc[:, s0 + GRP:s0 + 2 * GRP], in1=e1, op=Alu.add)
            nc.vector.scalar_tensor_tensor(
                out=colacc[:, s0 + 4096:s0 + 4224], in0=e2, scalar=0.5,
                in1=colacc[:, s0 + 4096:s0 + 4224],
                op0=Alu.mult, op1=Alu.add)

            # numerator, diag tile (f32): mask, diag extract, gate
            u0 = wp.tile([128, 128], f32, tag="u0")
            nc.vector.scalar_tensor_tensor(
                out=u0, in0=labw[:, s0:s0 + 128],
                scalar=lab_rows[:, t, :], in1=ed,
                op0=Alu.is_equal, op1=Alu.mult)
            scr = wp.tile([128, 128], f32, tag="scr")
            nc.vector.scalar_tensor_tensor(
                out=scr, in0=u0, scalar=1.0, in1=ident,
                op0=Alu.mult, op1=Alu.mult,
                accum_out=edacc[:, t:t + 1])
            nc.vector.tensor_tensor(out=u0, in0=u0, in1=anti, op=Alu.mult)
            scr2 = wp.tile([128, 128], f32, tag="scr2")
            nc.vector.scalar_tensor_tensor(
                out=scr2, in0=u0, scalar=1.0, in1=u0,
                op0=Alu.is_gt, op1=Alu.mult,
                accum_out=naccA[:, t:t + 1])
            # numerator, off-diag window (bf16): cols [s0+128, s0+384)
            u1 = wp.tile([128, 256], bf16, tag="u1")
            nc.vector.scalar_tensor_tensor(
                out=u1, in0=labw[:, s0 + 128:s0 + 384],
                scalar=lab_rows[:, t, :], in1=e0[:, 0:256],
                op0=Alu.is_equal, op1=Alu.mult)
            m1 = wp.tile([128, 256], bf16, tag="m1")
            nc.vector.scalar_tensor_tensor(
                out=m1, in0=u1, scalar=1.0, in1=u1,
                op0=Alu.is_gt, op1=Alu.mult,
                accum_out=naccB[:, t:t + 1])
            nc.vector.tensor_tensor(
                out=numcol[:, s0 + 128:s0 + 384],
                in0=numcol[:, s0 + 128:s0 + 384], in1=m1, op=Alu.add)

        # ---- epilogue: ship partials; host combines in fp64 ----
        nc.sync.dma_start(out=outd[:, 0:4 * NT], in_=dacc)
        nc.sync.dma_start(out=outd[:, 4 * NT:5 * NT], in_=naccA)
        nc.sync.dma_start(out=outd[:, 5 * NT:6 * NT], in_=naccB)
        nc.sync.dma_start(out=outd[:, 6 * NT:7 * NT], in_=edacc)
        nc.sync.dma_start(out=outcold.ap(), in_=colacc)
        nc.sync.dma_start(out=outnumd.ap(), in_=numcol)

    _split_excess_waits(nc)
    return nc


_NC = None


def _get_nc():
    global _NC
    if _NC is None:
        _NC = _build_nc()
    return _NC


def _host_reference(emb, lab):
    """Numpy fallback (only for pathological label distributions where a
    class exceeds the PAD margin; never triggers for the target regime)."""
    e = emb / np.linalg.norm(emb, axis=1, keepdims=True).astype(np.float32)
    sim = (e @ e.T).astype(np.float32) / np.float32(TEMP)
    E = np.exp(sim, dtype=np.float32)
    pos = (lab[:, None] == lab[None, :]) & ~np.eye(len(lab), dtype=bool)
    valid = pos & (sim > 0)
    num = np.where(valid, E, 0).sum(1, dtype=np.float32)
    den = E.sum(1, dtype=np.float32) - np.diagonal(E)
    rv = valid.any(1) & (den > 0)
    ns = np.where(rv, num, np.float32(1.0))
    ds = np.where(rv, den, np.float32(1.0))
    li = np.log(ds + np.float32(EPS)) - np.log(ns)
    nv = int(rv.sum())
    if nv == 0:
        return np.float32(0.0)
    return np.float32(abs(float(np.where(rv, li, 0).sum(dtype=np.float64)) / nv))


def kernel(**inputs):
    global LAST_RESULTS
    emb = np.ascontiguousarray(np.asarray(inputs["embeddings"], dtype=np.float32))
    lab = np.asarray(inputs["labels"]).astype(np.int64).ravel()
    assert emb.shape == (N, D) and lab.shape == (N,)

    if np.bincount(lab, minlength=1).max() > PAD:
        return _host_reference(emb, lab)

    _install_axon_trace_hook()

    # host prep: normalize, sort by label, per-core roll + transpose + fp8
    e = emb / np.linalg.norm(emb, axis=1, keepdims=True).astype(np.float32)
    order = np.argsort(lab, kind="stable")
    es = np.ascontiguousarray(e[order])
    ls = lab[order].astype(np.float32)

    anti = (1.0 - np.eye(128, dtype=np.float32)).astype(np.float32)
    ident = np.eye(128, dtype=np.float32)

    in_maps = []
    for c in range(M):
        shift = c * R - PAD
        rolled = np.roll(es, -shift, axis=0)         # [N, D] f32
        labr = (np.roll(ls, -shift) - LOFF).astype(ml_dtypes.bfloat16)
        # [D, N] fp8 -> two k-pair tensors [128, 2, N]
        qk = np.asarray((rolled.T * SCL).reshape(KT, 128, N),
                        dtype=ml_dtypes.float8_e4m3)
        q01 = np.ascontiguousarray(
            qk[0:2].transpose(1, 0, 2).reshape(128, 2 * N))
        q23 = np.ascontiguousarray(
            qk[2:4].transpose(1, 0, 2).reshape(128, 2 * N))
        in_maps.append({
            "qT01": q01,
            "qT23": q23,
            "lab": np.ascontiguousarray(labr.reshape(N, 1)),
            "anti": anti,
            "ident": ident,
        })

    nc = _get_nc()
    res = run_bass_kernel_spmd(nc, in_maps, core_ids=list(range(M)))
    LAST_RESULTS = res

    # host combine in fp64
    num_g = np.zeros(N)
    den_g = np.zeros(N)
    ed_g = np.zeros(N)
    for c in range(M):
        shift = c * R - PAD
        o = np.asarray(res.results[c]["out"], dtype=np.float64)    # [128, 56]
        csum = np.asarray(res.results[c]["outcol"], dtype=np.float64).sum(0)
        nsum = np.asarray(res.results[c]["outnum"], dtype=np.float64).sum(0)
        den_g[(np.arange(CW) + shift) % N] += csum
        num_g[(np.arange(NW) + shift) % N] += nsum
        dacc = o[:, 0:4 * NT]
        naccA = o[:, 4 * NT:5 * NT]
        naccB = o[:, 5 * NT:6 * NT]
        edacc = o[:, 6 * NT:7 * NT]
        for t in range(NT):
            s0 = PAD + 128 * t
            gr = (np.arange(s0, s0 + 128) + shift) % N
            den_g[gr] += (dacc[:, 4 * t] + dacc[:, 4 * t + 1]
                          + dacc[:, 4 * t + 2] + 0.5 * dacc[:, 4 * t + 3])
            num_g[gr] += naccA[:, t] + naccB[:, t]
            ed_g[gr] += edacc[:, t]
    den = den_g - ed_g
    rv = (num_g > 0) & (den > 0)
    nv = int(rv.sum())
    if nv == 0:
        return np.float32(0.0)
    li = np.log(np.where(rv, den, 1.0) + EPS) - np.log(np.where(rv, num_g, 1.0))
    return np.float32(abs(li[rv].sum() / nv))


# revision 11
# speedup vs baseline: 1.7596x; 1.1420x over previous
"""Contrastive-loss Trainium2 kernel: 8-way data-parallel, symmetric-half sims.

Strategy: rows are label-sorted; each of the 8 cores owns 8 row-tiles of 128
rows (its [1024] rows sit at fixed offset PAD in a per-core rolled layout).
Exploiting sim's symmetry, each row-tile computes only a 33-tile (4224-col)
strip starting at its diagonal tile: unordered pairs at circular tile
distance d are covered once for d<32, and the d=32 tile is computed by both
partners at weight 0.5 (folded into the exp as a ln(0.5) bias). Per strip:
fp8 DoubleRow matmuls (fp32 PSUM), exp on ScalarE with row-sums via
accum_out (denominator), one consolidated bf16 exp tile per strip added into
a column-sum buffer on DVE (the transposed halves of the denominator), and a
host-precomputed label-match mask gates the positive numerator on GPSIMD.
Per-core partials (row/col sums, diag sims) are combined on the host in fp64.
"""

import contextlib
import ctypes
import math
import os
import sys
import types

import ml_dtypes
import numpy as np

import concourse.bass as bass
import concourse.mybir as mybir
import concourse.tile as tile
from concourse.bass_utils import run_bass_kernel_spmd

# problem constants (hardcoded per task contract)
N, D, NCLS = 8192, 512, 512
TEMP = 0.07
EPS = 1e-8
M = 8            # cores
R = N // M       # 1024 rows per core
NT = R // 128    # 8 row-tiles per core
PAD = 128        # roll margin; must be >= max class size (else host fallback)
KT = D // 128    # 4 contraction tiles
GRP = 2048       # columns per psum group / exp call (4 banks)
ST = 33          # strip tiles per row-tile (d = 0..32; d=32 at weight 0.5)
SW = ST * 128    # 4224 strip width
QW = (NT - 1) * 128 + SW         # 5120 q cols needed: rolled [PAD, PAD+QW)
CW = PAD + (NT - 1) * 128 + SW   # 5248 colacc width; valid [2*PAD, CW)
MW = 256         # mask window per row-tile (diag tile + 1): needs PAD>=wmax
NW = PAD + (NT - 1) * 128 + MW   # 1280 numcol width; valid [2*PAD, NW)
SCL = 8.0        # fp8 pre-quantization scale (power of 2; avoids subnormals)
ESCALE = 1.0 / (SCL * SCL * TEMP)   # exp() input scale un-doing SCL^2

_AXON_SO = "/opt/axon/libaxon_pjrt.so"

LAST_RESULTS = None   # BassKernelResults of the most recent run (for test.py)


def _install_axon_trace_hook():
    """Provide antenv.axon_hooks (NTFF profiling) if the image lacks it."""
    try:
        from antenv.axon_hooks import get_axon_ntff_profile_hook  # noqa: F401
        return
    except ImportError:
        pass
    if not os.path.exists(_AXON_SO):
        return
    try:
        lib = ctypes.CDLL(_AXON_SO)
    except OSError:
        return
    if not hasattr(lib, "axon_start_nrt_profile"):
        return
    lib.axon_start_nrt_profile.argtypes = [ctypes.POINTER(ctypes.c_int64), ctypes.c_size_t]
    lib.axon_start_nrt_profile.restype = ctypes.c_int64
    lib.axon_stop_nrt_profile.argtypes = [ctypes.c_char_p]
    lib.axon_stop_nrt_profile.restype = ctypes.c_int64

    @contextlib.contextmanager
    def _hook(output_dir, device_ids):
        import jax
        jax.devices()
        if device_ids:
            ids = (ctypes.c_int64 * len(device_ids))(*device_ids)
            rc = lib.axon_start_nrt_profile(ids, len(device_ids))
        else:
            rc = lib.axon_start_nrt_profile(None, 0)
        if rc != 0:
            raise RuntimeError(f"axon_start_nrt_profile rc={rc}")
        try:
            yield
        finally:
            n = lib.axon_stop_nrt_profile(str(output_dir).encode())
            if n < 0:
                raise RuntimeError(f"axon_stop_nrt_profile rc={n}")

    _the_hook = [_hook]
    mod = types.ModuleType("antenv.axon_hooks")
    mod.set_axon_ntff_profile_hook = lambda h: _the_hook.__setitem__(0, h)
    mod.get_axon_ntff_profile_hook = lambda: _the_hook[0]
    sys.modules["antenv.axon_hooks"] = mod
    import antenv
    antenv.axon_hooks = mod


def _split_excess_waits(nc, max_waits=1):
    """This walrus build allows one sync-wait per instruction; move extras
    onto same-engine NoOps inserted just before (execution order preserved)."""
    for f in nc.m.functions:
        for b in f.blocks:
            insts = b.instructions
            new = []
            changed = False
            for inst in insts:
                si = inst.sync_info
                ow = list(si.on_wait) if (si and si.on_wait) else []
                if len(ow) > max_waits:
                    extra, keep = ow[:-max_waits], ow[-max_waits:]
                    for k, w in enumerate(extra):
                        nop = mybir.InstNoOp(name=f"{inst.name}-w{k}", ins=[], outs=[])
                        nop.engine = inst.engine
                        nop.sync_info = mybir.SyncInfo(on_wait=[w], on_update=[])
                        new.append(nop)
                    inst.sync_info = mybir.SyncInfo(
                        on_wait=keep,
                        on_update=list(si.on_update) if si.on_update else [])
                    changed = True
                new.append(inst)
            if changed:
                b.instructions = new


def _build_nc():
    f32 = mybir.dt.float32
    bf16 = mybir.dt.bfloat16
    f8 = mybir.dt.float8e4
    Alu = mybir.AluOpType
    Act = mybir.ActivationFunctionType
    DR = mybir.MatmulPerfMode.DoubleRow
    LNHALF = float(math.log(0.5))

    nc = bass.Bass(trn_type="TRN2", target_bir_lowering=False, debug=False)
    # k-subtile pairs split into two tensors so matmuls on pair 0 can start
    # once the first half of the embedding DMA lands
    qT01 = nc.dram_tensor("qT01", [128, 2 * QW], f8, kind="ExternalInput")
    qT23 = nc.dram_tensor("qT23", [128, 2 * QW], f8, kind="ExternalInput")
    maskd = nc.dram_tensor("maskh", [128, NT * MW], bf16, kind="ExternalInput")
    identd = nc.dram_tensor("ident", [128, 128], f32, kind="ExternalInput")
    # row-wise partials: dacc[3*NT] | nacc[NT] | sdiag[NT]
    outd = nc.dram_tensor("out", [128, 5 * NT], f32, kind="ExternalOutput")
    outcold = nc.dram_tensor("outcol", [128, CW], bf16, kind="ExternalOutput")
    outnumd = nc.dram_tensor("outnum", [128, NW], bf16, kind="ExternalOutput")

    with tile.TileContext(nc) as tc, contextlib.ExitStack() as ctx:
        qp = ctx.enter_context(tc.tile_pool(name="qp", bufs=1))
        pp = ctx.enter_context(tc.tile_pool(name="pp", bufs=2, space="PSUM"))
        ep = ctx.enter_context(tc.tile_pool(name="ep", bufs=2))
        wp = ctx.enter_context(tc.tile_pool(name="wp", bufs=2))
        sp = ctx.enter_context(tc.tile_pool(name="sp", bufs=1))

        # ---- preload ----
        qt01 = qp.tile([128, 2, QW], f8, tag="qt01")
        nc.sync.dma_start(out=qt01, in_=qT01[:, :])
        qt23 = qp.tile([128, 2, QW], f8, tag="qt23")
        nc.sync.dma_start(out=qt23, in_=qT23[:, :])
        mask = sp.tile([128, NT, MW], bf16)
        nc.sync.dma_start(out=mask, in_=maskd[:, :])
        ident = sp.tile([128, 128], f32)
        nc.sync.dma_start(out=ident, in_=identd.ap())
        lnhalf_t = sp.tile([128, 1], f32)
        nc.vector.memset(lnhalf_t, LNHALF)
        warm = sp.tile([128, 128], bf16)
        nc.vector.memset(warm, 0.0)
        warm_ps = pp.tile([128, GRP], f32, tag="ps")
        for w in range(48):
            nc.tensor.matmul(warm_ps[:, :128], warm, warm, start=True, stop=True)

        # ---- accumulators (colacc/numcol are copy-initialized, no memset) ----
        colacc = sp.tile([128, CW], bf16)    # exp col-sums (transposed den)
        numcol = sp.tile([128, NW], bf16)    # masked-pos col-sums (transposed num)
        dacc = sp.tile([128, 3 * NT], f32)   # exp row-sums: 3 group slots per t
        nacc = sp.tile([128, NT], f32)       # numerator row-sums
        sdia = sp.tile([128, NT], f32)       # diagonal sim (pre-exp) per t

        # ---- main loop: per row-tile, strip cols [q0, q0+SW) of qt ----
        for t in range(NT):
            q0 = 128 * t         # strip start in qt col coords
            s0 = PAD + 128 * t   # strip start in rolled col coords
            # group 0: cols [q0, q0+2048), k-pair sweeps to amortize ldweights
            ps0 = pp.tile([128, GRP], f32, tag="ps")
            for k, qt in ((0, qt01), (1, qt23)):
                for sub in range(4):
                    nc.tensor.matmul(
                        ps0[:, sub * 512:(sub + 1) * 512],
                        qt[:, :, q0:q0 + 128],
                        qt[:, :, q0 + sub * 512:q0 + (sub + 1) * 512],
                        start=(k == 0), stop=(k == 1), perf_mode=DR)
            e012 = ep.tile([128, SW], bf16, tag="e")
            nc.scalar.activation(
                out=e012[:, 0:GRP], in_=ps0[:], func=Act.Exp,
                scale=float(ESCALE), accum_out=dacc[:, 3 * t:3 * t + 1])
            # diag sims straight out of PSUM (fp32-exact; host exps them)
            scr = wp.tile([128, 128], f32, tag="scr")
            nc.vector.scalar_tensor_tensor(
                out=scr, in0=ps0[:, 0:128], scalar=1.0, in1=ident,
                op0=Alu.mult, op1=Alu.mult, accum_out=sdia[:, t:t + 1])
            # group 1: cols [q0+2048, q0+4096)
            ps1 = pp.tile([128, GRP], f32, tag="ps")
            for k, qt in ((0, qt01), (1, qt23)):
                for sub in range(4):
                    nc.tensor.matmul(
                        ps1[:, sub * 512:(sub + 1) * 512],
                        qt[:, :, q0:q0 + 128],
                        qt[:, :, q0 + GRP + sub * 512:q0 + GRP + (sub + 1) * 512],
                        start=(k == 0), stop=(k == 1), perf_mode=DR)
            nc.scalar.activation(
                out=e012[:, GRP:2 * GRP], in_=ps1[:], func=Act.Exp,
                scale=float(ESCALE), accum_out=dacc[:, 3 * t + 1:3 * t + 2])
            # group 2: cols [q0+4096, q0+4224), exp pre-halved via ln(0.5) bias
            ps2 = pp.tile([128, GRP], f32, tag="ps")
            for k, qt in ((0, qt01), (1, qt23)):
                nc.tensor.matmul(
                    ps2[:, 0:128],
                    qt[:, :, q0:q0 + 128],
                    qt[:, :, q0 + 4096:q0 + SW],
                    start=(k == 0), stop=(k == 1), perf_mode=DR)
            nc.scalar.activation(
                out=e012[:, 4096:SW], in_=ps2[:, 0:128], func=Act.Exp,
                scale=float(ESCALE), bias=lnhalf_t[:],
                accum_out=dacc[:, 3 * t + 2:3 * t + 3])

            # column accumulation (DVE): first touch is a copy, then adds
            if t == 0:
                nc.vector.tensor_copy(
                    out=colacc[:, s0 + 128:s0 + SW], in_=e012[:, 128:SW])
            else:
                nc.vector.tensor_tensor(
                    out=colacc[:, s0 + 128:s0 + 4096],
                    in0=colacc[:, s0 + 128:s0 + 4096],
                    in1=e012[:, 128:4096], op=Alu.add)
                nc.vector.tensor_copy(
                    out=colacc[:, s0 + 4096:s0 + SW], in_=e012[:, 4096:SW])

            # numerator (GPSIMD): host mask already zeroes the diagonal
            u0 = wp.tile([128, MW], bf16, tag="u0")
            nc.gpsimd.tensor_tensor(
                out=u0, in0=mask[:, t, :], in1=e012[:, 0:MW], op=Alu.mult)
            m0 = wp.tile([128, MW], bf16, tag="m0")
            nc.vector.scalar_tensor_tensor(
                out=m0, in0=u0, scalar=1.0, in1=u0,
                op0=Alu.is_gt, op1=Alu.mult,
                accum_out=nacc[:, t:t + 1])
            nc.gpsimd.tensor_copy(
                out=numcol[:, s0 + 128:s0 + MW], in_=m0[:, 128:MW])

        # ---- epilogue: ship partials; host combines in fp64 ----
        nc.sync.dma_start(out=outd[:, 0:3 * NT], in_=dacc)
        nc.sync.dma_start(out=outd[:, 3 * NT:4 * NT], in_=nacc)
        nc.sync.dma_start(out=outd[:, 4 * NT:5 * NT], in_=sdia)
        nc.sync.dma_start(out=outcold.ap(), in_=colacc)
        nc.sync.dma_start(out=outnumd.ap(), in_=numcol)

    _split_excess_waits(nc)
    return nc


_NC = None


def _get_nc():
    global _NC
    if _NC is None:
        _NC = _build_nc()
    return _NC


def _host_reference(emb, lab):
    """Numpy fallback (only for pathological label distributions where a
    class exceeds the PAD margin; never triggers for the target regime)."""
    e = emb / np.linalg.norm(emb, axis=1, keepdims=True).astype(np.float32)
    sim = (e @ e.T).astype(np.float32) / np.float32(TEMP)
    E = np.exp(sim, dtype=np.float32)
    pos = (lab[:, None] == lab[None, :]) & ~np.eye(len(lab), dtype=bool)
    valid = pos & (sim > 0)
    num = np.where(valid, E, 0).sum(1, dtype=np.float32)
    den = E.sum(1, dtype=np.float32) - np.diagonal(E)
    rv = valid.any(1) & (den > 0)
    ns = np.where(rv, num, np.float32(1.0))
    ds = np.where(rv, den, np.float32(1.0))
    li = np.log(ds + np.float32(EPS)) - np.log(ns)
    nv = int(rv.sum())
    if nv == 0:
        return np.float32(0.0)
    return np.float32(abs(float(np.where(rv, li, 0).sum(dtype=np.float64)) / nv))


def kernel(**inputs):
    global LAST_RESULTS
    emb = np.ascontiguousarray(np.asarray(inputs["embeddings"], dtype=np.float32))
    lab = np.asarray(inputs["labels"]).astype(np.int64).ravel()
    assert emb.shape == (N, D) and lab.shape == (N,)

    if np.bincount(lab, minlength=1).max() > PAD:
        return _host_reference(emb, lab)

    _install_axon_trace_hook()

    # host prep: normalize, sort by label, per-core roll + transpose + fp8
    e = emb / np.linalg.norm(emb, axis=1, keepdims=True).astype(np.float32)
    order = np.argsort(lab, kind="stable")
    es = np.ascontiguousarray(e[order])
    ls = lab[order]

    ident = np.eye(128, dtype=np.float32)
    noeye = 1.0 - np.eye(128, dtype=np.float32)

    in_maps = []
    for c in range(M):
        shift = c * R - PAD
        rolled = np.roll(es, -shift, axis=0)         # [N, D] f32
        labr = np.roll(ls, -shift)
        # [D, QW] fp8 (cols [PAD, PAD+QW)) -> two k-pair tensors [128, 2, QW]
        qk = np.asarray((rolled.T[:, PAD:PAD + QW] * SCL).reshape(KT, 128, QW),
                        dtype=ml_dtypes.float8_e4m3)
        q01 = np.ascontiguousarray(
            qk[0:2].transpose(1, 0, 2).reshape(128, 2 * QW))
        q23 = np.ascontiguousarray(
            qk[2:4].transpose(1, 0, 2).reshape(128, 2 * QW))
        # label-match mask per row-tile window, diagonal zeroed
        maskh = np.empty((128, NT * MW), dtype=np.float32)
        for t in range(NT):
            s0 = PAD + 128 * t
            m = (labr[s0:s0 + MW][None, :] == labr[s0:s0 + 128][:, None])
            m = m.astype(np.float32)
            m[:, 0:128] *= noeye
            maskh[:, t * MW:(t + 1) * MW] = m
        in_maps.append({
            "qT01": q01,
            "qT23": q23,
            "maskh": maskh.astype(ml_dtypes.bfloat16),
            "ident": ident,
        })

    nc = _get_nc()
    res = run_bass_kernel_spmd(nc, in_maps, core_ids=list(range(M)))
    LAST_RESULTS = res

    # host combine in fp64
    num_g = np.zeros(N)
    den_g = np.zeros(N)
    for c in range(M):
        shift = c * R - PAD
        o = np.asarray(res.results[c]["out"], dtype=np.float64)    # [128, 5*NT]
        csum = np.asarray(
            res.results[c]["outcol"][:, 2 * PAD:], dtype=np.float64).sum(0)
        nsum = np.asarray(
            res.results[c]["outnum"][:, 2 * PAD:], dtype=np.float64).sum(0)
        den_g[(np.arange(2 * PAD, CW) + shift) % N] += csum
        num_g[(np.arange(2 * PAD, NW) + shift) % N] += nsum
        dacc = o[:, 0:3 * NT]
        nacc = o[:, 3 * NT:4 * NT]
        sdia = o[:, 4 * NT:5 * NT]
        for t in range(NT):
            s0 = PAD + 128 * t
            gr = (np.arange(s0, s0 + 128) + shift) % N
            den_g[gr] += (dacc[:, 3 * t] + dacc[:, 3 * t + 1]
                          + dacc[:, 3 * t + 2] - np.exp(ESCALE * sdia[:, t]))
            num_g[gr] += nacc[:, t]
    rv = (num_g > 0) & (den_g > 0)
    nv = int(rv.sum())
    if nv == 0:
        return np.float32(0.0)
    li = np.log(np.where(rv, den_g, 1.0) + EPS) - np.log(np.where(rv, num_g, 1.0))
    return np.float32(abs(li[rv].sum() / nv))
